# revision 1
# baseline (speedup 1.0000x reference)
"""Trainium2 Bass kernel for nn_Decoder (dense transformer, B=8,T=512,D=1024,H=16,L=4,C=10).

Sharding: data-parallel over batch — core b computes batch element b end-to-end.
Weights (~25M params, bf16) are streamed per core; no collectives.

v3: software-pipelined attention (next chunk's q/k matmuls fill the PE while
ACT runs exp of the current head), one 2048-wide exp per head from a 4-bank
PSUM score tile, interleaved bn_stats (computed right after each residual
segment lands so LN boundaries don't stall the PE), bf16 weights + bf16
transposes, causal mask as an f16 multiply on the exp output, quarter-tile
streamed head overlapping the last FFN.
"""
import sys
sys.path.insert(0, '/opt/trn_rl_repo')

import numpy as np

T, D, H, DH, L, C = 512, 1024, 16, 64, 4, 10
NT, ND = T // 128, D // 128          # 4 t-tiles, 8 d-tiles
NG = H // 4                          # 4 head groups == weight chunks
EPS = 1e-5
ISCALE = 1.0 / 32.0                  # 1/sqrt(D)
MAGIC = 0x5F3759DF                   # rsqrt Newton seed

_cache = {}


def _build(has_gb, reps=1):
    import concourse.bass as bass
    from concourse import bacc, tile, mybir
    from concourse.masks import make_identity

    f32 = mybir.dt.float32
    f32r = mybir.dt.float32r
    bf16 = mybir.dt.bfloat16
    f16 = mybir.dt.float16
    i32 = mybir.dt.int32
    AF = mybir.ActivationFunctionType
    AL = mybir.AluOpType

    nc = bacc.Bacc("TRN2", target_bir_lowering=False, debug=False, num_devices=8)

    x_d = nc.dram_tensor("x", [T, D], f32, kind="ExternalInput").ap()
    xbf_d = nc.dram_tensor("xbf", [T, D], bf16, kind="ExternalInput").ap()
    ktc_d = nc.dram_tensor("ktc", [D, T], bf16, kind="ExternalInput").ap()
    vac_d = nc.dram_tensor("vac", [T, H * 65], f16, kind="ExternalInput").ap()
    w_d = {
        m: nc.dram_tensor(f"w{m}", [L, 4, 128, ND * 256], bf16,
                          kind="ExternalInput").ap()
        for m in ("q1", "k1", "v1", "q2", "fc")
    }
    bfc_d = nc.dram_tensor("bfc", [L, D], f32r, kind="ExternalInput").ap()
    bcut_d = nc.dram_tensor("bcut", [C - 1], f32, kind="ExternalInput").ap()
    if has_gb:
        gb_d = nc.dram_tensor("gb", [3 * L - 1, 2, D], f32, kind="ExternalInput").ap()
    probs_d = nc.dram_tensor("probs", [T, D, C], f32, kind="ExternalOutput").ap()

    def bcast(src_ap, parts=128):
        return bass.AP(tensor=src_ap.tensor, offset=src_ap.offset,
                       ap=[[0, parts]] + list(src_ap.ap))

    with tile.TileContext(nc) as tc:
        with tc.tile_pool(name="P1", bufs=1) as P1, \
             tc.tile_pool(name="Pw", bufs=6) as Pw, \
             tc.tile_pool(name="Pst", bufs=2) as Pst, \
             tc.tile_pool(name="Psm", bufs=4) as Psm, \
             tc.tile_pool(name="Pgb", bufs=2) as Pgb, \
             tc.tile_pool(name="Phd", bufs=5) as Phd, \
             tc.tile_pool(name="Pps", bufs=2, space="PSUM") as Pps:

            # ---------- persistent tiles ----------
            x_res = P1.tile([128, NT, D], f32)
            x_bf = P1.tile([128, NT, D], bf16)
            xT = P1.tile([128, ND, T], bf16)
            qT = P1.tile([128, ND, T], bf16)      # reused for q2T
            kT1 = P1.tile([128, ND, T], bf16)
            kTc = P1.tile([128, ND, T], bf16)
            vc = P1.tile([128, NT, H, 65], f16)
            v1 = P1.tile([128, NT, H, 65], f16)
            identb = P1.tile([128, 128], bf16)
            maskb = P1.tile([128, 128], f16)   # 1 where q >= k else 0
            bcut_sb = P1.tile([128, C - 1], f32)
            magic_t = P1.tile([128, 1], i32)
            nc.vector.memset(magic_t, MAGIC)
            ones1f = P1.tile([1, 128], f32)
            nc.vector.memset(ones1f, 1.0)
            ones1 = P1.tile([1, 128], f32r)
            nc.vector.tensor_copy(ones1, ones1f)

            make_identity(nc, identb)
            # keep (1.0) where q >= k, zero where q < k
            nc.gpsimd.memset(maskb, 1.0)
            nc.gpsimd.affine_select(
                out=maskb, in_=maskb, compare_op=AL.is_ge, fill=0.0,
                base=0, pattern=[[1, 128]], channel_multiplier=-1)
            nc.vector.memset(v1[:, :, :, 64:65], 1.0)

            # ---------- loop-invariant loads (gpsimd queue) ----------
            nc.gpsimd.dma_start(kTc, ktc_d.rearrange("(jt p) t -> p jt t", p=128))
            nc.gpsimd.dma_start(vc, vac_d.rearrange("(tk p) j -> p tk j", p=128)
                                .rearrange("p tk (h e) -> p tk h e", h=H))
            nc.gpsimd.dma_start(bcut_sb, bcast(bcut_d))

            gb_idx = [0]
            st_cur = [None]    # stats tile for the LN fed by the current phase

            def st_new():
                st_cur[0] = Psm.tile([128, NT, 4, 6], f32, tag="st", bufs=2,
                                     name="st")
                return st_cur[0]

            def ln_finish():
                """Aggregate interleaved stats, Newton rsqrt, apply LN:
                x_bf (bf16, gates transposes) first, then x_res in place."""
                st = st_cur[0]
                if has_gb:
                    g_b = Pgb.tile([128, D], f32, tag="g_b")
                    b_b = Pgb.tile([128, D], f32, tag="b_b")
                    nc.sync.dma_start(g_b, bcast(gb_d[gb_idx[0] % (3 * L - 1), 0, :]))
                    nc.sync.dma_start(b_b, bcast(gb_d[gb_idx[0] % (3 * L - 1), 1, :]))
                mvall = Psm.tile([128, NT, 2], f32, tag="mvall", name="mvall")
                for t in range(NT):
                    nc.vector.bn_aggr(mvall[:, t, :], st[:, t, :, :])
                v1t = Psm.tile([128, NT], f32, tag="v1t")
                nc.vector.tensor_scalar_add(v1t, mvall[:, :, 1], EPS)
                sh = Psm.tile([128, NT], i32, tag="sh")
                nc.vector.tensor_scalar(sh, v1t.bitcast(i32), 1, None,
                                        AL.arith_shift_right)
                magic_b = bass.AP(tensor=magic_t.tensor, offset=magic_t.offset,
                                  ap=[magic_t.ap[0], [0, NT]])
                y = Psm.tile([128, NT], f32, tag="y")
                nc.vector.tensor_tensor(out=y.bitcast(i32), in0=magic_b, in1=sh,
                                        op=AL.subtract)
                for _ in range(2):
                    a = Psm.tile([128, NT], f32, tag="nta")
                    nc.vector.tensor_tensor(out=a, in0=y, in1=y, op=AL.mult)
                    b = Psm.tile([128, NT], f32, tag="ntb")
                    nc.vector.tensor_tensor(out=b, in0=a, in1=v1t, op=AL.mult)
                    c2 = Psm.tile([128, NT], f32, tag="ntc")
                    nc.vector.tensor_scalar(c2, b, -0.5, 1.5, AL.mult, AL.add)
                    y2 = Psm.tile([128, NT], f32, tag="y", name="y2")
                    nc.vector.tensor_tensor(out=y2, in0=y, in1=c2, op=AL.mult)
                    y = y2
                for t in range(NT):
                    if has_gb:
                        nc.vector.tensor_scalar(
                            x_res[:, t, :], x_res[:, t, :],
                            mvall[:, t, 0:1], y[:, t:t + 1], AL.subtract, AL.mult)
                        nc.vector.scalar_tensor_tensor(
                            out=x_res[:, t, :], in0=x_res[:, t, :], scalar=0.0,
                            in1=g_b, op0=AL.bypass, op1=AL.mult)
                        nc.vector.tensor_tensor(
                            out=x_res[:, t, :], in0=x_res[:, t, :], in1=b_b,
                            op=AL.add)
                        nc.vector.tensor_copy(x_bf[:, t, :], x_res[:, t, :])
                    else:
                        # bf16 copy on DVE (gates the transposes); the f32
                        # in-place update goes to the idle GPSIMD
                        nc.vector.tensor_scalar(
                            x_bf[:, t, :], x_res[:, t, :],
                            mvall[:, t, 0:1], y[:, t:t + 1], AL.subtract, AL.mult)
                        nc.vector.tensor_scalar(
                            x_res[:, t, :], x_res[:, t, :],
                            mvall[:, t, 0:1], y[:, t:t + 1], AL.subtract, AL.mult)
                gb_idx[0] += 1

            def build_xT():
                """xT (bf16) <- transpose(x_bf) via PE in bf16. Copies split
                DVE/ACT so neither serializes the phase prologue."""
                for dt in range(ND):
                    ps = Pps.tile([128, 512], bf16, tag="psL", bufs=2,
                                  name="psT")
                    for t in range(NT):
                        nc.tensor.transpose(
                            ps[:, t * 128:(t + 1) * 128],
                            x_bf[:, t, dt * 128:(dt + 1) * 128], identb)
                        if t % 2 == 1:
                            dst = xT[:, dt, (t - 1) * 128:(t + 1) * 128]
                            src = ps[:, (t - 1) * 128:(t + 1) * 128]
                            if dt % 2 == 0:
                                nc.vector.tensor_copy(dst, src)
                            else:
                                nc.scalar.copy(dst, src)

            def wdma(w, wsrc, l, ch, eng):
                eng.dma_start(w.rearrange("p dt j -> p (dt j)"), wsrc[l, ch])

            def lin_piece(w, jt, dest, copy_eng):
                """One [128, T] column block of (x @ W.T).T into dest[jt]."""
                j2 = jt % 2
                ps = Pps.tile([128, 512], f32, tag="psL", bufs=2, name="psL")
                for dt in range(ND):
                    nc.tensor.matmul(
                        ps, w[:, dt, j2 * 128:(j2 + 1) * 128],
                        xT[:, dt, :], start=(dt == 0), stop=(dt == ND - 1))
                if copy_eng is nc.scalar:
                    nc.scalar.copy(dest[:, jt, :], ps)
                else:
                    copy_eng.tensor_copy(dest[:, jt, :], ps)

            def qk_exp(hg, hh, kT, expg, causal):
                """Scores for head 4hg+hh into a 4-bank psum tile, one
                2048-wide exp into expg; causal pads get exp(stale) that AV
                never reads, diag blocks masked by an f16 multiply."""
                h = hg * 4 + hh
                po = (h % 2) * 64
                jt = h // 2
                sc = Pps.tile([128, NT, 512], f32, tag="sc", bufs=1, name="sc")
                for tk in range(NT):
                    tq0 = tk * 128 if causal else 0
                    nc.tensor.matmul(
                        sc[:, tk, tq0:T],
                        kT[po:po + 64, jt, tk * 128:(tk + 1) * 128],
                        qT[po:po + 64, jt, tq0:T],
                        start=True, stop=True)
                nc.scalar.activation(expg[:, hh, :, :], sc, AF.Exp,
                                     scale=ISCALE)
                if causal:
                    eh = expg[:, hh, 0, 0:128]
                    diag = bass.AP(tensor=eh.tensor, offset=eh.offset,
                                   ap=[eh.ap[0], [T + 128, NT], [1, 128]])
                    mask_b = bass.AP(tensor=maskb.tensor, offset=maskb.offset,
                                     ap=[maskb.ap[0], [0, NT], [1, 128]])
                    nc.vector.tensor_tensor(out=diag, in0=diag, in1=mask_b,
                                            op=AL.mult)

            def av_out(hg, expg, vv, causal, st_dst):
                """AV, normalize, accumulate into x_res; bn_stats for the
                next LN interleaved right after each residual segment."""
                for tqi in range(NT):
                    tq = (tqi + hg) % NT
                    ntk = (tq + 1) if causal else NT
                    op = Pps.tile([128, 4, 65], f32, tag="ogrp", bufs=2,
                                  name="psO")
                    for hh in range(4):
                        h = hg * 4 + hh
                        for tk in range(ntk):
                            nc.tensor.matmul(
                                op[:, hh, :],
                                expg[:, hh, tk, tq * 128:(tq + 1) * 128],
                                vv[:, tk, h, :],
                                start=(tk == 0), stop=(tk == ntk - 1))
                    rec = Psm.tile([128, 4], f32, tag="rec", bufs=4,
                                   name="rec")
                    nc.vector.reciprocal(rec, op[:, :, 64])
                    rec_b = bass.AP(tensor=rec.tensor, offset=rec.offset,
                                    ap=[rec.ap[0], rec.ap[1], [0, 64]])
                    onrm = Pst.tile([128, 4, 64], f32, tag="onrm",
                                    name="onrm")
                    nc.vector.scalar_tensor_tensor(
                        out=onrm, in0=op[:, :, 0:64], scalar=0.0,
                        in1=rec_b, op0=AL.bypass, op1=AL.mult)
                    seg = x_res[:, tq, hg * 256:(hg + 1) * 256]
                    nc.vector.scalar_tensor_tensor(
                        out=seg, in0=onrm.rearrange("p h e -> p (h e)"),
                        scalar=0.0, in1=seg, op0=AL.bypass, op1=AL.add)
                    if st_dst is not None:
                        nc.vector.bn_stats(st_dst[:, tq, hg, :], seg)

            def head_tile(t):
                """Ordinal sigmoid head for t-tile t, quarter-tile staging so
                stores stream out while later tiles compute."""
                xs = x_res[:, t, :]
                prs = [Phd.tile([128, 256, C], f32, tag="pr", bufs=5,
                                name="pr") for _ in range(4)]
                sprev = None
                for c in range(C - 1):
                    scur = Pst.tile([128, D], f32, tag="sig", bufs=3,
                                    name="sig")
                    nc.scalar.activation(scur, xs, AF.Sigmoid, scale=-1.0,
                                         bias=bcut_sb[:, c:c + 1])
                    for q in range(4):
                        eng = nc.gpsimd if q == 3 else nc.vector
                        sc_q = scur[:, q * 256:(q + 1) * 256]
                        if c == 0:
                            eng.tensor_copy(prs[q][:, :, 0], sc_q)
                        else:
                            eng.tensor_tensor(
                                out=prs[q][:, :, c], in0=sc_q,
                                in1=sprev[:, q * 256:(q + 1) * 256],
                                op=AL.subtract)
                    sprev = scur
                pd = probs_d.rearrange("(t p) (q dd) c -> t q p dd c",
                                       p=128, q=4)
                for q in range(4):
                    eng = nc.gpsimd if q == 3 else nc.vector
                    eng.tensor_scalar(prs[q][:, :, C - 1],
                                      sprev[:, q * 256:(q + 1) * 256],
                                      -1.0, 1.0, AL.mult, AL.add)
                    (nc.sync if q % 2 == 0 else nc.gpsimd).dma_start(
                        pd[t, q], prs[q])

            def self_attn(l):
                """Self-attention, software-pipelined: while ACT runs exp of
                head group hg, the PE computes q1/k1 pieces of hg+1."""
                wq = [None] * NG
                wk = [None] * NG
                wv = [None] * NG

                def load_qk(g):
                    wq[g] = Pw.tile([128, ND, 256], bf16, tag="w", name="wq")
                    wdma(wq[g], w_d["q1"], l, g, nc.sync)
                    wk[g] = Pw.tile([128, ND, 256], bf16, tag="w", name="wk")
                    wdma(wk[g], w_d["k1"], l, g, nc.sync)

                def load_v(g):
                    wv[g] = Pw.tile([128, ND, 256], bf16, tag="w", name="wv")
                    wdma(wv[g], w_d["v1"], l, g, nc.gpsimd)

                load_qk(0)
                load_v(0)
                for jt in (0, 1):
                    lin_piece(wq[0], jt, qT, nc.scalar)
                    lin_piece(wk[0], jt, kT1, nc.scalar)
                st = st_new()
                for hg in range(NG):
                    if hg + 1 < NG:
                        load_qk(hg + 1)
                        load_v(hg + 1)
                    expg = Pst.tile([128, 4, NT, T], f16, tag="expg",
                                    name="expg")
                    for hh in range(4):
                        qk_exp(hg, hh, kT1, expg, causal=False)
                        # fill PE while exp(hh) runs on ACT
                        if hg + 1 < NG:
                            dest, w, jt = ((qT, wq[hg + 1], 2 * (hg + 1) + hh % 2)
                                           if hh < 2 else
                                           (kT1, wk[hg + 1], 2 * (hg + 1) + hh % 2))
                            lin_piece(w, jt, dest, nc.scalar)
                        else:
                            t = hh
                            ps = Pps.tile([128, 512], f32, tag="psL", bufs=2,
                                          name="psV")
                            for dt in range(ND):
                                nc.tensor.matmul(
                                    ps[:, 0:256],
                                    xT[:, dt, t * 128:(t + 1) * 128],
                                    wv[hg][:, dt, :], start=(dt == 0),
                                    stop=(dt == ND - 1))
                            nc.vector.tensor_copy(
                                v1[:, t, hg * 4:(hg + 1) * 4, 0:64],
                                ps[:, 0:256].rearrange("p (h e) -> p h e", e=64))
                    if hg + 1 < NG:
                        for t in range(NT):
                            ps = Pps.tile([128, 512], f32, tag="psL", bufs=2,
                                          name="psV")
                            for dt in range(ND):
                                nc.tensor.matmul(
                                    ps[:, 0:256],
                                    xT[:, dt, t * 128:(t + 1) * 128],
                                    wv[hg][:, dt, :], start=(dt == 0),
                                    stop=(dt == ND - 1))
                            nc.vector.tensor_copy(
                                v1[:, t, hg * 4:(hg + 1) * 4, 0:64],
                                ps[:, 0:256].rearrange("p (h e) -> p h e", e=64))
                    av_out(hg, expg, v1, causal=False, st_dst=st)

            def cross_attn(l):
                wq = [None] * NG

                def load_q(g):
                    wq[g] = Pw.tile([128, ND, 256], bf16, tag="w", name="wq2")
                    wdma(wq[g], w_d["q2"], l, g, nc.sync)

                load_q(0)
                for jt in (0, 1):
                    lin_piece(wq[0], jt, qT, nc.vector)
                st = st_new()
                for hg in range(NG):
                    if hg + 1 < NG:
                        load_q(hg + 1)
                    expg = Pst.tile([128, 4, NT, T], f16, tag="expg",
                                    name="expg")
                    for hh in range(4):
                        qk_exp(hg, hh, kTc, expg, causal=True)
                        if hg + 1 < NG and hh < 2:
                            lin_piece(wq[hg + 1], 2 * (hg + 1) + hh, qT,
                                      nc.vector)
                    av_out(hg, expg, vc, causal=True, st_dst=st)

            def ffn(l):
                bfc_row = Pgb.tile([1, D], f32r, tag="bfc_row",
                                   name="bfc_row")
                nc.gpsimd.dma_start(
                    bfc_row,
                    bass.AP(tensor=bfc_d.tensor, offset=bfc_d[l].offset,
                            ap=[[0, 1], [1, D]]))
                st = st_new() if l + 1 < L else None
                for ch in range(4):
                    w = Pw.tile([128, ND, 256], bf16, tag="w", name="wF")
                    wdma(w, w_d["fc"], l, ch, nc.sync)
                    for t in range(NT):
                        ps = Pps.tile([128, 512], f32, tag="psL", bufs=2,
                                      name="psF")
                        for dt in range(ND):
                            nc.tensor.matmul(
                                ps[:, 0:256],
                                xT[:, dt, t * 128:(t + 1) * 128],
                                w[:, dt, :], start=(dt == 0), stop=False)
                        nc.tensor.matmul(
                            ps[:, 0:256], ones1,
                            bfc_row[:, ch * 256:(ch + 1) * 256],
                            start=False, stop=True)
                        seg = x_res[:, t, ch * 256:(ch + 1) * 256]
                        nc.vector.scalar_tensor_tensor(
                            out=seg, in0=ps[:, 0:256], scalar=0.0,
                            in1=seg, op0=AL.max, op1=AL.add)
                        if st is not None:
                            nc.vector.bn_stats(st[:, t, ch, :], seg)
                        if l == L - 1 and ch == 3:
                            head_tile(t)

            def body():
                nc.sync.dma_start(x_res, x_d.rearrange("(t p) d -> p t d", p=128))
                nc.gpsimd.dma_start(x_bf, xbf_d.rearrange("(t p) d -> p t d", p=128))
                for l in range(L):
                    if l > 0:
                        ln_finish()
                    build_xT()
                    self_attn(l)
                    ln_finish()
                    build_xT()
                    cross_attn(l)
                    ln_finish()
                    build_xT()
                    ffn(l)

            if reps > 1:
                with tc.For_i(0, reps, 1):
                    body()
            else:
                body()

    nc.compile()
    return nc


def _prep(inputs):
    import ml_dtypes
    x = np.asarray(inputs["x"])
    k = np.asarray(inputs["k"])
    v = np.asarray(inputs["v"])
    pos = np.asarray(inputs["pos"])
    B = x.shape[0]

    xp = (x + pos[None]).astype(np.float32)                       # [B,T,D]
    xbf = xp.astype(ml_dtypes.bfloat16)
    ktc = np.ascontiguousarray(
        k.transpose(0, 1, 3, 2).reshape(B, H * DH, T)).astype(ml_dtypes.bfloat16)
    va = np.ones((B, T, H, 65), np.float16)
    va[..., :64] = v.transpose(0, 2, 1, 3)                        # [B,tk,h,e]
    va = va.reshape(B, T, H * 65)

    wt = {}
    for name, key in (("q1", "Wq1"), ("k1", "Wk1"), ("v1", "Wv1"),
                      ("q2", "Wq2"), ("fc", "Wfc")):
        # W [L, j, d] -> W.T [L, d, j] -> chunked [L, ch, p, (dt jj)]
        wT = np.asarray(inputs[key]).transpose(0, 2, 1).astype(np.float32)
        wc = wT.reshape(L, ND, 128, 4, 256).transpose(0, 3, 2, 1, 4)
        wt[name] = np.ascontiguousarray(
            wc.reshape(L, 4, 128, ND * 256)).astype(ml_dtypes.bfloat16)

    cut = np.asarray(inputs["cutoff"]).astype(np.float32)
    bcut = np.cumsum(
        np.concatenate([cut[:, :1], cut[:, 1:] ** 2], axis=1), axis=1)[0]  # [9]

    g1, b1 = np.asarray(inputs["g1"]), np.asarray(inputs["b1"])
    g2, b2 = np.asarray(inputs["g2"]), np.asarray(inputs["b2"])
    g3, b3 = np.asarray(inputs["g3"]), np.asarray(inputs["b3"])
    trivial = (np.all(g1 == 1) and np.all(g2 == 1) and np.all(g3 == 1)
               and np.all(b1 == 0) and np.all(b2 == 0) and np.all(b3 == 0))
    gb = None
    if not trivial:
        rows = []
        for l in range(L):
            if l > 0:
                rows.append((g1[l - 1], b1[l - 1]))
            rows.append((g2[l], b2[l]))
            rows.append((g3[l], b3[l]))
        gb = np.stack([np.stack(r) for r in rows]).astype(np.float32)

    bfc = np.asarray(inputs["bfc"]).astype(np.float32)
    return xp, xbf, ktc, va, wt, bfc, bcut, gb, B


def kernel(**inputs):
    from concourse.bass_utils import run_bass_kernel_spmd

    xp, xbf, ktc, va, wt, bfc, bcut, gb, B = _prep(inputs)
    has_gb = gb is not None
    if ("nc", has_gb) not in _cache:
        _cache[("nc", has_gb)] = _build(has_gb)
    nc = _cache[("nc", has_gb)]

    in_maps = []
    for b in range(B):
        m = {
            "x": xp[b], "xbf": xbf[b], "ktc": ktc[b], "vac": va[b],
            "wq1": wt["q1"], "wk1": wt["k1"], "wv1": wt["v1"],
            "wq2": wt["q2"], "wfc": wt["fc"],
            "bfc": bfc, "bcut": bcut,
        }
        if has_gb:
            m["gb"] = gb
        in_maps.append(m)

    res = run_bass_kernel_spmd(nc, in_maps, list(range(B)))
    out = np.stack([res.results[b]["probs"] for b in range(B)])
    return out.astype(np.float32)

